# revision 22
# baseline (speedup 1.0000x reference)
"""Trainium2 Bass kernel: MoE-routed multi-head attention (nn_MultiHeadAttention_80204219286041).

Sharding: data-parallel over batch B=8 -> one sample per NeuronCore (8 cores).
Expert tables are replicated; everything per-sample is computed on-core with no
collectives.  Host slices inputs per core and reassembles (out, attn).

Per-core dataflow (L=1024, D=512, H=8, dk=dv=64, E=8, F=H*dk=512):
  1. gate  = sigmoid(fc2(relu(fc1(mean_l x_))))                 [1, E]
  2. Wmix  = sum_e gate[e] * W[e]  (PE scaled-identity accum)   [D, F] x3
     (1/sqrt(D) is folded into Wmix_q.)
  3. qT,kT,vT = PE transposes of q,k,v                          [D, L]
     qsT = Wmix_q^T @ q = (lhsT=Wmix_q, rhs=qT)                 [F, L] bf16
     ksT likewise; vs (natural [L, F]) goes straight into vs_aug (bf16)
     with a ones column appended per head -> PV matmul also yields softmax Z.
  4. per head h:
       scoresT[j,i] (lhsT=ksT_h, rhs=qsT_h) -> ACT exp -> expT bf16
       PV: (lhsT=vs_aug[jc,h], rhs=expT) accum -> [dv+1, i]; row dv is Z_i
       headT = PV[0:dv] * (1/Z) (broadcast via DRAM-bounce DMA)
       scores natural [i,j] (operands swapped) -> ACT exp -> * (1/Z_i) -> attn out
  5. out = LN(headT^T @ woT + wo_b + q) * g + b   (bn_stats/bn_aggr per row)
"""

import sys

if "/opt/trn_rl_repo" not in sys.path:
    sys.path.insert(0, "/opt/trn_rl_repo")

import numpy as np

import concourse.bass as bass
import concourse.bacc as bacc
import concourse.tile as tile
from concourse import mybir
from concourse.bass import MemorySpace
from concourse.masks import make_identity
from concourse.bass_utils import run_bass_kernel_spmd

B, L, D, H, DK, DV, E = 8, 1024, 512, 8, 64, 64, 8
F = H * DK  # 512
P = 128
LC = L // P  # 8 l-chunks
DC = D // P  # 4 d-chunks
FC = F // P  # 4 f-chunks
JC = L // P  # 8 j-chunks
NI = 2       # i halves of 512
LN_EPS = 1e-5
INV_SQRT_D = float(1.0 / np.sqrt(np.float32(D)))

FP = mybir.dt.float32
BF = mybir.dt.bfloat16
AF = mybir.ActivationFunctionType
ALU = mybir.AluOpType

_NC = None
LAST_RESULTS = None  # exposed for test harness (exec_time_ns etc.)


def _body(nc, tc, t_in, out_o, attn_o):
    from contextlib import ExitStack
    with ExitStack() as ctx:
        _body_inner(nc, tc, ctx, t_in, out_o, attn_o)


def _body_inner(nc, tc, ctx, t_in, out_o, attn_o):
    consts = ctx.enter_context(tc.tile_pool(name="consts", bufs=1))
    persist = ctx.enter_context(tc.tile_pool(name="persist", bufs=1))
    dram = ctx.enter_context(tc.tile_pool(name="dram", bufs=2, space=MemorySpace.DRAM))

    identity = consts.tile([P, P], FP)
    make_identity(nc, identity)
    ones = consts.tile([P, P], FP)
    nc.gpsimd.memset(ones, 1.0)
    ones_bf = consts.tile([P, 1], BF)
    nc.gpsimd.memset(ones_bf, 1.0)
    eps_t = consts.tile([P, 1], FP)
    nc.gpsimd.memset(eps_t, LN_EPS)
    # broadcast [D]-vectors across partitions once
    g_bc = consts.tile([P, D], FP)
    nc.sync.dma_start(out=g_bc, in_=t_in["ln_g"][None, :].to_broadcast([P, D]))
    b_bc = consts.tile([P, D], FP)
    nc.sync.dma_start(out=b_bc, in_=t_in["ln_b"][None, :].to_broadcast([P, D]))
    wob_bc = consts.tile([P, D], FP)
    nc.sync.dma_start(out=wob_bc, in_=t_in["wo_b"][None, :].to_broadcast([P, D]))

    # persistent activations
    q_nat = persist.tile([P, LC, D], FP)       # q natural, for residual + transpose src
    qsT = persist.tile([P, FC, L], BF)         # q-proj, feature-major
    ksT = persist.tile([P, FC, L], BF)
    vs_aug = persist.tile([P, JC, H, DV + 1], BF)  # v-proj per (jc, h) + ones col
    headT = persist.tile([P, FC, L], BF)       # normalized attention output, feature-major
    idg = persist.tile([P, E, P], BF)          # gate-scaled identities

    # ---------------- Phase A: gate ----------------
    with tc.tile_pool(name="ph_a", bufs=2) as pa, \
         tc.tile_pool(name="ph_a_ps", bufs=4, space=MemorySpace.PSUM) as pa_ps, \
         tc.tile_pool(name="ph_a_ps2", bufs=2, space=MemorySpace.PSUM) as pa_ps2:
        # pooled as a row vector (ones^T @ x chunks), then DMA-bounce transpose
        ps_pr = pa_ps.tile([1, D], FP, tag="pooled")
        for lc in range(LC):
            xt = pa.tile([P, D], BF, tag=f"xt{lc}")
            nc.sync.dma_start(out=xt, in_=t_in["x_"][lc * P:(lc + 1) * P, :])
            nc.tensor.matmul(ps_pr, lhsT=ones_bf, rhs=xt,
                             start=(lc == 0), stop=(lc == LC - 1))
        pr_sb = pa.tile([1, D], FP)
        nc.vector.tensor_scalar_mul(pr_sb, ps_pr, 1.0 / L)
        pr_d = dram.tile([1, D], FP, tag="pr_d")
        nc.gpsimd.dma_start(out=pr_d, in_=pr_sb)
        pooledT = pa.tile([P, DC], FP)
        nc.gpsimd.dma_start(out=pooledT,
                            in_=pr_d.rearrange("o (c p) -> (o p) c", p=P))

        f1n = pa.tile([E, D], FP)
        nc.sync.dma_start(out=f1n, in_=t_in["fc1_w"][:, :])
        ps_f1t = pa_ps2.tile([P, DC, E], FP, tag="tiny")
        for dc in range(DC):
            nc.tensor.transpose(ps_f1t[:, dc, :], f1n[0:E, dc * P:(dc + 1) * P],
                                identity[0:E, 0:E])
        f1t = pa.tile([P, DC, E], FP)
        nc.vector.tensor_copy(f1t, ps_f1t)

        ps_h = pa_ps2.tile([1, E], FP, tag="tiny")
        for dc in range(DC):
            nc.tensor.matmul(ps_h, lhsT=pooledT[:, dc:dc + 1], rhs=f1t[:, dc, :],
                             start=(dc == 0), stop=(dc == DC - 1))
        f1b = pa.tile([1, E], FP)
        nc.sync.dma_start(out=f1b, in_=t_in["fc1_b"][None, :])
        h_sb = pa.tile([1, E], FP)
        nc.vector.tensor_add(h_sb, ps_h[0:1, :], f1b)
        nc.vector.tensor_scalar_max(h_sb, h_sb, 0.0)

        ps_hT = pa_ps2.tile([E, 1], FP, tag="tiny")
        nc.tensor.matmul(ps_hT, lhsT=h_sb[0:1, :], rhs=ones[0:1, 0:1],
                         start=True, stop=True)
        hT = pa.tile([E, 1], FP)
        nc.vector.tensor_copy(hT, ps_hT)

        f2n = pa.tile([E, E], FP)
        nc.sync.dma_start(out=f2n, in_=t_in["fc2_w"][:, :])
        ps_f2t = pa_ps2.tile([E, E], FP, tag="tiny")
        nc.tensor.transpose(ps_f2t, f2n[0:E, 0:E], identity[0:E, 0:E])
        f2t = pa.tile([E, E], FP)
        nc.vector.tensor_copy(f2t, ps_f2t)

        ps_g = pa_ps2.tile([1, E], FP, tag="tiny")
        nc.tensor.matmul(ps_g, lhsT=hT[0:E, 0:1], rhs=f2t[0:E, :], start=True, stop=True)
        f2b = pa.tile([1, E], FP)
        nc.sync.dma_start(out=f2b, in_=t_in["fc2_b"][None, :])
        gpre = pa.tile([1, E], FP)
        nc.vector.tensor_add(gpre, ps_g[0:1, :], f2b)
        gate_sb = pa.tile([1, E], FP)
        nc.scalar.activation(gate_sb, gpre, AF.Sigmoid)

        ps_gb = pa_ps2.tile([P, E], FP, tag="tiny")
        nc.tensor.matmul(ps_gb, lhsT=ones[0:1, :], rhs=gate_sb[0:1, :],
                         start=True, stop=True)
        gate_bc = pa.tile([P, E], FP)
        nc.vector.tensor_copy(gate_bc, ps_gb)
        for e in range(E):
            nc.vector.tensor_scalar_mul(idg[:, e, :], identity, gate_bc[:, e:e + 1])

    # ---------------- Phase B: load pre-transposed q,k,v and woT ----------------
    # mid-lifetime pool: released after phase D to free SBUF for attention
    mid = tc.alloc_tile_pool(name="mid", bufs=1)
    qT = mid.tile([P, DC, L], BF)
    kT = mid.tile([P, DC, L], BF)
    vT = mid.tile([P, DC, L], BF)
    woT = persist.tile([P, FC, D], BF)
    for lc in range(LC):
        nc.sync.dma_start(out=q_nat[:, lc, :],
                          in_=t_in["q"][lc * P:(lc + 1) * P, :])
    for src_name, dstT in (("qT", qT), ("kT", kT), ("vT", vT)):
        for dc in range(DC):
            nc.sync.dma_start(out=dstT[:, dc, :],
                              in_=t_in[src_name][dc * P:(dc + 1) * P, :])
    for t in range(FC):
        nc.sync.dma_start(out=woT[:, t, :],
                          in_=t_in["woT"][t * P:(t + 1) * P, :])

    # ---------------- Phase C: Wmix = sum_e gate[e] W[e] ----------------
    wmq = mid.tile([P, DC, F], BF)
    wmk = mid.tile([P, DC, F], BF)
    wmv = mid.tile([P, DC, F], BF)
    with tc.tile_pool(name="wload", bufs=10) as wl, \
         tc.tile_pool(name="wm_ps", bufs=2, space=MemorySpace.PSUM) as wm_ps:
        for w_name, wm, scale in (("Wq", wmq, INV_SQRT_D), ("Wk", wmk, 1.0),
                                  ("Wv", wmv, 1.0)):
            for dc in range(DC):
                psw = wm_ps.tile([P, F], FP, tag="psw")
                for e in range(E):
                    wt = wl.tile([P, F], BF, tag="wt")
                    nc.sync.dma_start(out=wt,
                                      in_=t_in[w_name][e, dc * P:(dc + 1) * P, :])
                    nc.tensor.matmul(psw, lhsT=idg[:, e, :], rhs=wt,
                                     start=(e == 0), stop=(e == E - 1))
                if scale == 1.0:
                    nc.vector.tensor_copy(wm[:, dc, :], psw)
                else:
                    nc.vector.tensor_scalar_mul(wm[:, dc, :], psw, scale)

    # ---------------- Phase D: projections ----------------
    with tc.tile_pool(name="pj_ps", bufs=4, space=MemorySpace.PSUM) as pj_ps:
        for wm, xT, dstT in ((wmq, qT, qsT), (wmk, kT, ksT)):
            for fc in range(FC):
                for lh in range(NI):
                    psp = pj_ps.tile([P, L // NI], FP, tag="psp")
                    for dc in range(DC):
                        nc.tensor.matmul(
                            psp,
                            lhsT=wm[:, dc, fc * P:(fc + 1) * P],
                            rhs=xT[:, dc, lh * (L // NI):(lh + 1) * (L // NI)],
                            start=(dc == 0), stop=(dc == DC - 1))
                    nc.vector.tensor_copy(
                        dstT[:, fc, lh * (L // NI):(lh + 1) * (L // NI)], psp)
        # vs natural [L, F] -> straight into vs_aug (+ ones column via memset)
        nc.gpsimd.memset(vs_aug, 1.0)
        for jc in range(JC):
            psv = pj_ps.tile([P, F], FP, tag="psv")
            for dc in range(DC):
                nc.tensor.matmul(psv, lhsT=vT[:, dc, jc * P:(jc + 1) * P],
                                 rhs=wmv[:, dc, :],
                                 start=(dc == 0), stop=(dc == DC - 1))
            for h in range(H):
                nc.vector.tensor_copy(vs_aug[:, jc, h, 0:DV],
                                      psv[:, h * DV:(h + 1) * DV])
    mid.release()

    # ---------------- Phase F: attention per head ----------------
    with tc.tile_pool(name="att", bufs=3) as att, \
         tc.tile_pool(name="att_s", bufs=4) as att_s, \
         tc.tile_pool(name="ps_s", bufs=2, space=MemorySpace.PSUM) as ps_s, \
         tc.tile_pool(name="ps_pv", bufs=2, space=MemorySpace.PSUM) as ps_pv, \
         tc.tile_pool(name="ps_n", bufs=1, space=MemorySpace.PSUM) as ps_n:
        NIW = L // NI  # 512

        def emit_nat_pc(hh, rct_h, pc):
            # natural-layout scores -> exp -> *1/Z -> normalized probs out
            po_, fc_ = (hh % 2) * DK, hh // 2
            psn = ps_n.tile([P, L], FP, tag="psn")
            for jh in range(NI):
                nc.tensor.matmul(
                    psn[:, jh * NIW:(jh + 1) * NIW],
                    lhsT=qsT[po_:po_ + DK, fc_, pc * P:(pc + 1) * P],
                    rhs=ksT[po_:po_ + DK, fc_, jh * NIW:(jh + 1) * NIW],
                    start=True, stop=True)
            pn_bf = att_s.tile([P, L], BF, tag="pn_bf")
            nc.scalar.activation(pn_bf, psn, AF.Exp)
            pf = att_s.tile([P, L], FP, tag="pf")
            nc.gpsimd.tensor_scalar_mul(pf, pn_bf, rct_h[:, pc:pc + 1])
            nc.sync.dma_start(out=attn_o[hh, pc * P:(pc + 1) * P, :], in_=pf)

        prev = None  # (h, rct) pending natural-layout output pass
        for h in range(H):
            po = (h % 2) * DK
            fc = h // 2
            expT = att.tile([P, JC, L], BF, tag="expT")
            # interleave: scoresT/exp for this head, PV one step behind, and
            # the previous head's output pass -- keeps PE and ACT streams dense
            pspvs = [ps_pv.tile([DV + 1, NIW], FP, tag="pspv", name=f"pspv{ic}")
                     for ic in range(NI)]

            def emit_pv(jc):
                for ic in range(NI):
                    nc.tensor.matmul(pspvs[ic], lhsT=vs_aug[:, jc, h, :],
                                     rhs=expT[:, jc, ic * NIW:(ic + 1) * NIW],
                                     start=(jc == 0), stop=(jc == JC - 1))

            for step in range(JC):
                jc = step
                pss = ps_s.tile([P, L], FP, tag="pss")
                for ic in range(NI):
                    nc.tensor.matmul(
                        pss[:, ic * NIW:(ic + 1) * NIW],
                        lhsT=ksT[po:po + DK, fc, jc * P:(jc + 1) * P],
                        rhs=qsT[po:po + DK, fc, ic * NIW:(ic + 1) * NIW],
                        start=True, stop=True)
                nc.scalar.activation(expT[:, jc, :], pss, AF.Exp)
                if step >= 1:
                    emit_pv(step - 1)
                if prev is not None:
                    emit_nat_pc(prev[0], prev[1], step)
            emit_pv(JC - 1)
            prev = None
            z_sb = att_s.tile([1, L], FP, tag="z_sb")
            for ic in range(NI):
                nc.vector.tensor_copy(z_sb[0:1, ic * NIW:(ic + 1) * NIW],
                                      pspvs[ic][DV:DV + 1, :])
            # Z row -> (via DRAM bounce) transposed [P, LC] -> 1/Z both as
            # per-partition scalars (rct) and broadcast rows (rb)
            z_d = dram.tile([1, L], FP, tag="z_d")
            nc.gpsimd.dma_start(out=z_d, in_=z_sb)
            zt = att_s.tile([P, LC], FP, tag="zt")
            nc.gpsimd.dma_start(out=zt, in_=z_d.rearrange("o (c p) -> (o p) c", p=P))
            rct = att_s.tile([P, LC], FP, tag="rct")
            nc.vector.reciprocal(rct, zt)
            # scatter recipT back to a row-major DRAM row so rb loads are
            # contiguous-broadcast DMAs
            rr_d = dram.tile([1, L], FP, tag="rr_d")
            nc.gpsimd.dma_start(out=rr_d.rearrange("o (c p) -> (o p) c", p=P),
                                in_=rct)
            for ic in range(NI):
                rb = att_s.tile([DV, NIW], FP, tag="rb")
                nc.gpsimd.dma_start(
                    out=rb,
                    in_=rr_d[0:1, ic * NIW:(ic + 1) * NIW].to_broadcast([DV, NIW]))
                nc.vector.tensor_mul(headT[po:po + DK, fc, ic * NIW:(ic + 1) * NIW],
                                     pspvs[ic][0:DV, :], rb)
            prev = (h, rct)
        for pc in range(LC):
            emit_nat_pc(prev[0], prev[1], pc)

    # ---------------- Phase G: output projection + residual + LN ----------------
    with tc.tile_pool(name="ph_g", bufs=3) as pg, \
         tc.tile_pool(name="ph_g_ps", bufs=2, space=MemorySpace.PSUM) as pg_ps:
        for lc in range(LC):
            ps_o = pg_ps.tile([P, D], FP, tag="ps_o")
            for t in range(FC):
                nc.tensor.matmul(ps_o, lhsT=headT[:, t, lc * P:(lc + 1) * P],
                                 rhs=woT[:, t, :], start=(t == 0), stop=(t == FC - 1))
            t1 = pg.tile([P, D], FP, tag="t1")
            nc.vector.tensor_add(t1, ps_o, q_nat[:, lc, :])
            t1b = pg.tile([P, D], FP, tag="t1b")
            nc.vector.tensor_add(t1b, t1, wob_bc)
            stats = pg.tile([P, nc.vector.BN_STATS_DIM], FP, tag="stats")
            nc.vector.bn_stats(out=stats, in_=t1b)
            mv = pg.tile([P, nc.vector.BN_AGGR_DIM], FP, tag="mv")
            nc.vector.bn_aggr(out=mv, in_=stats)
            sd = pg.tile([P, 1], FP, tag="sd")
            nc.scalar.activation(sd, mv[:, 1:2], AF.Sqrt, bias=eps_t)
            rstd = pg.tile([P, 1], FP, tag="rstd")
            nc.vector.reciprocal(rstd, sd)
            t2 = pg.tile([P, D], FP, tag="t2")
            nc.vector.tensor_scalar(out=t2, in0=t1b, scalar1=mv[:, 0:1],
                                    scalar2=rstd, op0=ALU.subtract, op1=ALU.mult)
            t3 = pg.tile([P, D], FP, tag="t3")
            nc.vector.tensor_mul(t3, t2, g_bc)
            t4 = pg.tile([P, D], FP, tag="t4")
            nc.vector.tensor_add(t4, t3, b_bc)
            nc.sync.dma_start(out=out_o[lc * P:(lc + 1) * P, :], in_=t4)


def _build():
    nc = bacc.Bacc()
    t_in = {}
    specs = [
        ("q", [L, D], FP), ("x_", [L, D], BF),
        ("qT", [D, L], BF), ("kT", [D, L], BF), ("vT", [D, L], BF),
        ("Wq", [E, D, F], BF), ("Wk", [E, D, F], BF), ("Wv", [E, D, F], BF),
        ("fc1_w", [E, D], FP), ("fc1_b", [E], FP),
        ("fc2_w", [E, E], FP), ("fc2_b", [E], FP),
        ("woT", [F, D], BF), ("wo_b", [D], FP),
        ("ln_g", [D], FP), ("ln_b", [D], FP),
    ]
    for name, shape, dt in specs:
        t_in[name] = nc.declare_dram_parameter(name, shape, dt, isOutput=False)
    out_o = nc.declare_dram_parameter("out", [L, D], FP, isOutput=True)
    attn_o = nc.declare_dram_parameter("attn", [H, L, L], FP, isOutput=True)
    with tile.TileContext(nc) as tc:
        _body(nc, tc, t_in, out_o, attn_o)
    nc.compile()
    return nc


def _get_nc():
    global _NC
    if _NC is None:
        _NC = _build()
    return _NC


def kernel(q, k, v, x_, Wq, Wk, Wv, fc1_w, fc1_b, fc2_w, fc2_b, wo_w, wo_b,
           ln_g, ln_b, **run_kwargs):
    global LAST_RESULTS
    import ml_dtypes
    BF_NP = ml_dtypes.bfloat16
    nc = _get_nc()
    f32 = lambda a: np.ascontiguousarray(np.asarray(a, dtype=np.float32))
    bf = lambda a: np.ascontiguousarray(np.asarray(a, dtype=np.float32).astype(BF_NP))
    shared = {
        "Wq": bf(Wq), "Wk": bf(Wk), "Wv": bf(Wv),
        "fc1_w": f32(fc1_w), "fc1_b": f32(fc1_b),
        "fc2_w": f32(fc2_w), "fc2_b": f32(fc2_b),
        "woT": bf(np.asarray(wo_w, np.float32).T), "wo_b": f32(wo_b),
        "ln_g": f32(ln_g), "ln_b": f32(ln_b),
    }
    q = f32(q)
    k32, v32, x32 = (np.asarray(a, np.float32) for a in (k, v, x_))
    in_maps = []
    for b in range(B):
        m = dict(shared)
        m["q"] = q[b]
        m["qT"] = bf(q[b].T)
        m["kT"] = bf(k32[b].T)
        m["vT"] = bf(v32[b].T)
        m["x_"] = bf(x32[b])
        in_maps.append(m)
    res = run_bass_kernel_spmd(nc, in_maps, list(range(B)), **run_kwargs)
    LAST_RESULTS = res
    out = np.empty((B, L, D), dtype=np.float32)
    attn = np.empty((H * B, L, L), dtype=np.float32)
    for b in range(B):
        out[b] = res.results[b]["out"]
        a = res.results[b]["attn"]  # [H, L, L]
        for h in range(H):
            attn[h * B + b] = a[h]
    return out, attn


if __name__ == "__main__":
    rng = np.random.default_rng(0)
    ins = {
        "q": rng.standard_normal((B, L, D), dtype=np.float32),
        "k": rng.standard_normal((B, L, D), dtype=np.float32),
        "v": rng.standard_normal((B, L, D), dtype=np.float32),
        "x_": rng.standard_normal((B, L, D), dtype=np.float32),
        "Wq": 0.02 * rng.standard_normal((E, D, F), dtype=np.float32),
        "Wk": 0.02 * rng.standard_normal((E, D, F), dtype=np.float32),
        "Wv": 0.02 * rng.standard_normal((E, D, F), dtype=np.float32),
        "fc1_w": 0.02 * rng.standard_normal((E, D), dtype=np.float32),
        "fc1_b": np.zeros(E, np.float32),
        "fc2_w": 0.02 * rng.standard_normal((E, E), dtype=np.float32),
        "fc2_b": np.zeros(E, np.float32),
        "wo_w": 0.02 * rng.standard_normal((D, F), dtype=np.float32),
        "wo_b": np.zeros(D, np.float32),
        "ln_g": np.ones(D, np.float32),
        "ln_b": np.zeros(D, np.float32),
    }
    out, attn = kernel(**ins)
    print("out", out.shape, out.dtype, "attn", attn.shape, attn.dtype)
    print("attn row sums ~1:", attn.sum(-1).mean())


# revision 23
# speedup vs baseline: 2.7544x; 2.7544x over previous
"""Trainium2 Bass kernel: MoE-routed multi-head attention (nn_MultiHeadAttention_80204219286041).

Sharding: data-parallel over batch B=8 -> one sample per NeuronCore (8 cores).
Expert tables are replicated; everything per-sample is computed on-core with no
collectives.  Host slices inputs per core and reassembles (out, attn).

Per-core dataflow (L=1024, D=512, H=8, dk=dv=64, E=8, F=H*dk=512):
  1. gate  = sigmoid(fc2(relu(fc1(mean_l x_))))                 [1, E]
  2. Wmix  = sum_e gate[e] * W[e]  (PE scaled-identity accum)   [D, F] x3
     (1/sqrt(D) is folded into Wmix_q.)
  3. qT,kT,vT = PE transposes of q,k,v                          [D, L]
     qsT = Wmix_q^T @ q = (lhsT=Wmix_q, rhs=qT)                 [F, L] bf16
     ksT likewise; vs (natural [L, F]) goes straight into vs_aug (bf16)
     with a ones column appended per head -> PV matmul also yields softmax Z.
  4. per head h:
       scoresT[j,i] (lhsT=ksT_h, rhs=qsT_h) -> ACT exp -> expT bf16
       PV: (lhsT=vs_aug[jc,h], rhs=expT) accum -> [dv+1, i]; row dv is Z_i
       headT = PV[0:dv] * (1/Z) (broadcast via DRAM-bounce DMA)
       scores natural [i,j] (operands swapped) -> ACT exp -> * (1/Z_i) -> attn out
  5. out = LN(headT^T @ woT + wo_b + q) * g + b   (bn_stats/bn_aggr per row)
"""

import sys

if "/opt/trn_rl_repo" not in sys.path:
    sys.path.insert(0, "/opt/trn_rl_repo")

import numpy as np

import concourse.bass as bass
import concourse.bacc as bacc
import concourse.tile as tile
from concourse import mybir
from concourse.bass import MemorySpace
from concourse.masks import make_identity
from concourse.bass_utils import run_bass_kernel_spmd

B, L, D, H, DK, DV, E = 8, 1024, 512, 8, 64, 64, 8
F = H * DK  # 512
P = 128
LC = L // P  # 8 l-chunks
DC = D // P  # 4 d-chunks
FC = F // P  # 4 f-chunks
JC = L // P  # 8 j-chunks
NI = 2       # i halves of 512
LN_EPS = 1e-5
INV_SQRT_D = float(1.0 / np.sqrt(np.float32(D)))

FP = mybir.dt.float32
BF = mybir.dt.bfloat16
AF = mybir.ActivationFunctionType
ALU = mybir.AluOpType

_NC = None
LAST_RESULTS = None  # exposed for test harness (exec_time_ns etc.)


def _body(nc, tc, t_in, out_o, attn_o):
    from contextlib import ExitStack
    with ExitStack() as ctx:
        _body_inner(nc, tc, ctx, t_in, out_o, attn_o)


def _body_inner(nc, tc, ctx, t_in, out_o, attn_o):
    consts = ctx.enter_context(tc.tile_pool(name="consts", bufs=1))
    persist = ctx.enter_context(tc.tile_pool(name="persist", bufs=1))
    dram = ctx.enter_context(tc.tile_pool(name="dram", bufs=2, space=MemorySpace.DRAM))

    identity = consts.tile([P, P], FP)
    make_identity(nc, identity)
    ones = consts.tile([P, P], FP)
    nc.gpsimd.memset(ones, 1.0)
    ones_bf = consts.tile([P, 1], BF)
    nc.gpsimd.memset(ones_bf, 1.0)
    eps_t = consts.tile([P, 1], FP)
    nc.gpsimd.memset(eps_t, LN_EPS)
    # broadcast [D]-vectors across partitions once
    g_bc = consts.tile([P, D], FP)
    nc.sync.dma_start(out=g_bc, in_=t_in["ln_g"][None, :].to_broadcast([P, D]))
    b_bc = consts.tile([P, D], FP)
    nc.sync.dma_start(out=b_bc, in_=t_in["ln_b"][None, :].to_broadcast([P, D]))
    wob_bc = consts.tile([P, D], FP)
    nc.sync.dma_start(out=wob_bc, in_=t_in["wo_b"][None, :].to_broadcast([P, D]))

    # persistent activations
    q_nat = persist.tile([P, LC, D], FP)       # q natural, for residual + transpose src
    qsT = persist.tile([P, FC, L], BF)         # q-proj, feature-major
    ksT = persist.tile([P, FC, L], BF)
    vs_aug = persist.tile([P, JC, H, DV + 1], BF)  # v-proj per (jc, h) + ones col
    headT = persist.tile([P, FC, L], BF)       # normalized attention output, feature-major
    idg = persist.tile([P, E, P], BF)          # gate-scaled identities

    # ---------------- Phase A: gate ----------------
    with tc.tile_pool(name="ph_a", bufs=2) as pa, \
         tc.tile_pool(name="ph_a_ps", bufs=4, space=MemorySpace.PSUM) as pa_ps, \
         tc.tile_pool(name="ph_a_ps2", bufs=2, space=MemorySpace.PSUM) as pa_ps2:
        # pooled as a row vector (ones^T @ x chunks), then DMA-bounce transpose
        ps_pr = pa_ps.tile([1, D], FP, tag="pooled")
        for lc in range(LC):
            xt = pa.tile([P, D], BF, tag=f"xt{lc}")
            nc.sync.dma_start(out=xt, in_=t_in["x_"][lc * P:(lc + 1) * P, :])
            nc.tensor.matmul(ps_pr, lhsT=ones_bf, rhs=xt,
                             start=(lc == 0), stop=(lc == LC - 1))
        pr_sb = pa.tile([1, D], FP)
        nc.vector.tensor_scalar_mul(pr_sb, ps_pr, 1.0 / L)
        pr_d = dram.tile([1, D], FP, tag="pr_d")
        nc.gpsimd.dma_start(out=pr_d, in_=pr_sb)
        pooledT = pa.tile([P, DC], FP)
        nc.gpsimd.dma_start(out=pooledT,
                            in_=pr_d.rearrange("o (c p) -> (o p) c", p=P))

        f1n = pa.tile([E, D], FP)
        nc.sync.dma_start(out=f1n, in_=t_in["fc1_w"][:, :])
        ps_f1t = pa_ps2.tile([P, DC, E], FP, tag="tiny")
        for dc in range(DC):
            nc.tensor.transpose(ps_f1t[:, dc, :], f1n[0:E, dc * P:(dc + 1) * P],
                                identity[0:E, 0:E])
        f1t = pa.tile([P, DC, E], FP)
        nc.vector.tensor_copy(f1t, ps_f1t)

        ps_h = pa_ps2.tile([1, E], FP, tag="tiny")
        for dc in range(DC):
            nc.tensor.matmul(ps_h, lhsT=pooledT[:, dc:dc + 1], rhs=f1t[:, dc, :],
                             start=(dc == 0), stop=(dc == DC - 1))
        f1b = pa.tile([1, E], FP)
        nc.sync.dma_start(out=f1b, in_=t_in["fc1_b"][None, :])
        h_sb = pa.tile([1, E], FP)
        nc.vector.tensor_add(h_sb, ps_h[0:1, :], f1b)
        nc.vector.tensor_scalar_max(h_sb, h_sb, 0.0)

        ps_hT = pa_ps2.tile([E, 1], FP, tag="tiny")
        nc.tensor.matmul(ps_hT, lhsT=h_sb[0:1, :], rhs=ones[0:1, 0:1],
                         start=True, stop=True)
        hT = pa.tile([E, 1], FP)
        nc.vector.tensor_copy(hT, ps_hT)

        f2n = pa.tile([E, E], FP)
        nc.sync.dma_start(out=f2n, in_=t_in["fc2_w"][:, :])
        ps_f2t = pa_ps2.tile([E, E], FP, tag="tiny")
        nc.tensor.transpose(ps_f2t, f2n[0:E, 0:E], identity[0:E, 0:E])
        f2t = pa.tile([E, E], FP)
        nc.vector.tensor_copy(f2t, ps_f2t)

        ps_g = pa_ps2.tile([1, E], FP, tag="tiny")
        nc.tensor.matmul(ps_g, lhsT=hT[0:E, 0:1], rhs=f2t[0:E, :], start=True, stop=True)
        f2b = pa.tile([1, E], FP)
        nc.sync.dma_start(out=f2b, in_=t_in["fc2_b"][None, :])
        gpre = pa.tile([1, E], FP)
        nc.vector.tensor_add(gpre, ps_g[0:1, :], f2b)
        gate_sb = pa.tile([1, E], FP)
        nc.scalar.activation(gate_sb, gpre, AF.Sigmoid)

        ps_gb = pa_ps2.tile([P, E], FP, tag="tiny")
        nc.tensor.matmul(ps_gb, lhsT=ones[0:1, :], rhs=gate_sb[0:1, :],
                         start=True, stop=True)
        gate_bc = pa.tile([P, E], FP)
        nc.vector.tensor_copy(gate_bc, ps_gb)
        for e in range(E):
            nc.vector.tensor_scalar_mul(idg[:, e, :], identity, gate_bc[:, e:e + 1])

    # ---------------- Phase B: load pre-transposed q,k,v and woT ----------------
    # mid-lifetime pool: released after phase D to free SBUF for attention
    mid = tc.alloc_tile_pool(name="mid", bufs=1)
    qT = mid.tile([P, DC, L], BF)
    kT = mid.tile([P, DC, L], BF)
    vT = mid.tile([P, DC, L], BF)
    woT = persist.tile([P, FC, D], BF)
    for lc in range(LC):
        nc.sync.dma_start(out=q_nat[:, lc, :],
                          in_=t_in["q"][lc * P:(lc + 1) * P, :])
    for src_name, dstT in (("qT", qT), ("kT", kT), ("vT", vT)):
        for dc in range(DC):
            nc.sync.dma_start(out=dstT[:, dc, :],
                              in_=t_in[src_name][dc * P:(dc + 1) * P, :])
    for t in range(FC):
        nc.sync.dma_start(out=woT[:, t, :],
                          in_=t_in["woT"][t * P:(t + 1) * P, :])

    # ---------------- Phase C: Wmix = sum_e gate[e] W[e] ----------------
    wmq = mid.tile([P, DC, F], BF)
    wmk = mid.tile([P, DC, F], BF)
    wmv = mid.tile([P, DC, F], BF)
    with tc.tile_pool(name="wload", bufs=10) as wl, \
         tc.tile_pool(name="wm_ps", bufs=2, space=MemorySpace.PSUM) as wm_ps:
        for w_name, wm, scale in (("Wq", wmq, INV_SQRT_D), ("Wk", wmk, 1.0),
                                  ("Wv", wmv, 1.0)):
            for dc in range(DC):
                psw = wm_ps.tile([P, F], FP, tag="psw")
                for e in range(E):
                    wt = wl.tile([P, F], BF, tag="wt")
                    nc.sync.dma_start(out=wt,
                                      in_=t_in[w_name][e, dc * P:(dc + 1) * P, :])
                    nc.tensor.matmul(psw, lhsT=idg[:, e, :], rhs=wt,
                                     start=(e == 0), stop=(e == E - 1))
                if scale == 1.0:
                    nc.vector.tensor_copy(wm[:, dc, :], psw)
                else:
                    nc.vector.tensor_scalar_mul(wm[:, dc, :], psw, scale)

    # ---------------- Phase D: projections ----------------
    with tc.tile_pool(name="pj_ps", bufs=4, space=MemorySpace.PSUM) as pj_ps:
        for wm, xT, dstT in ((wmq, qT, qsT), (wmk, kT, ksT)):
            for fc in range(FC):
                for lh in range(NI):
                    psp = pj_ps.tile([P, L // NI], FP, tag="psp")
                    for dc in range(DC):
                        nc.tensor.matmul(
                            psp,
                            lhsT=wm[:, dc, fc * P:(fc + 1) * P],
                            rhs=xT[:, dc, lh * (L // NI):(lh + 1) * (L // NI)],
                            start=(dc == 0), stop=(dc == DC - 1))
                    nc.vector.tensor_copy(
                        dstT[:, fc, lh * (L // NI):(lh + 1) * (L // NI)], psp)
        # vs natural [L, F] -> straight into vs_aug (+ ones column via memset)
        nc.gpsimd.memset(vs_aug, 1.0)
        for jc in range(JC):
            psv = pj_ps.tile([P, F], FP, tag="psv")
            for dc in range(DC):
                nc.tensor.matmul(psv, lhsT=vT[:, dc, jc * P:(jc + 1) * P],
                                 rhs=wmv[:, dc, :],
                                 start=(dc == 0), stop=(dc == DC - 1))
            for h in range(H):
                nc.vector.tensor_copy(vs_aug[:, jc, h, 0:DV],
                                      psv[:, h * DV:(h + 1) * DV])
    mid.release()

    # ---------------- Phase F: attention per head ----------------
    with tc.tile_pool(name="att", bufs=3) as att, \
         tc.tile_pool(name="att_s", bufs=3) as att_s, \
         tc.tile_pool(name="ps_s", bufs=2, space=MemorySpace.PSUM) as ps_s, \
         tc.tile_pool(name="ps_pv", bufs=2, space=MemorySpace.PSUM) as ps_pv, \
         tc.tile_pool(name="ps_n", bufs=1, space=MemorySpace.PSUM) as ps_n:
        NIW = L // NI  # 512

        def emit_nat_pc(hh, rct_h, pc):
            # natural-layout scores -> exp -> *1/Z -> normalized probs out
            po_, fc_ = (hh % 2) * DK, hh // 2
            psn = ps_n.tile([P, L], FP, tag="psn")
            for jh in range(NI):
                nc.tensor.matmul(
                    psn[:, jh * NIW:(jh + 1) * NIW],
                    lhsT=qsT[po_:po_ + DK, fc_, pc * P:(pc + 1) * P],
                    rhs=ksT[po_:po_ + DK, fc_, jh * NIW:(jh + 1) * NIW],
                    start=True, stop=True)
            pn_bf = att_s.tile([P, L], BF, tag="pn_bf")
            nc.scalar.activation(pn_bf, psn, AF.Exp)
            pf = att_s.tile([P, L], FP, tag="pf")
            nc.vector.tensor_scalar_mul(pf, pn_bf, rct_h[:, pc:pc + 1])
            nc.sync.dma_start(out=attn_o[hh, pc * P:(pc + 1) * P, :], in_=pf)

        prev = None  # (h, rct) pending natural-layout output pass
        for h in range(H):
            po = (h % 2) * DK
            fc = h // 2
            expT = att.tile([P, JC, L], BF, tag="expT")
            # interleave: scoresT/exp for this head, PV one step behind, and
            # the previous head's output pass -- keeps PE and ACT streams dense
            pspvs = [ps_pv.tile([DV + 1, NIW], FP, tag="pspv", name=f"pspv{ic}")
                     for ic in range(NI)]

            def emit_pv(jc):
                for ic in range(NI):
                    nc.tensor.matmul(pspvs[ic], lhsT=vs_aug[:, jc, h, :],
                                     rhs=expT[:, jc, ic * NIW:(ic + 1) * NIW],
                                     start=(jc == 0), stop=(jc == JC - 1))

            for step in range(JC):
                jc = step
                pss = ps_s.tile([P, L], FP, tag="pss")
                for ic in range(NI):
                    nc.tensor.matmul(
                        pss[:, ic * NIW:(ic + 1) * NIW],
                        lhsT=ksT[po:po + DK, fc, jc * P:(jc + 1) * P],
                        rhs=qsT[po:po + DK, fc, ic * NIW:(ic + 1) * NIW],
                        start=True, stop=True)
                nc.scalar.activation(expT[:, jc, :], pss, AF.Exp)
                if step >= 1:
                    emit_pv(step - 1)
                if prev is not None:
                    emit_nat_pc(prev[0], prev[1], step)
            emit_pv(JC - 1)
            prev = None
            z_sb = att_s.tile([1, L], FP, tag="z_sb")
            for ic in range(NI):
                nc.vector.tensor_copy(z_sb[0:1, ic * NIW:(ic + 1) * NIW],
                                      pspvs[ic][DV:DV + 1, :])
            # Z row -> (via DRAM bounce) transposed [P, LC] -> 1/Z both as
            # per-partition scalars (rct) and broadcast rows (rb)
            z_d = dram.tile([1, L], FP, tag="z_d")
            nc.gpsimd.dma_start(out=z_d, in_=z_sb)
            zt = att_s.tile([P, LC], FP, tag="zt")
            nc.gpsimd.dma_start(out=zt, in_=z_d.rearrange("o (c p) -> (o p) c", p=P))
            rct = att_s.tile([P, LC], FP, tag="rct")
            nc.vector.reciprocal(rct, zt)
            # scatter recipT back to a row-major DRAM row so rb loads are
            # contiguous-broadcast DMAs
            rr_d = dram.tile([1, L], FP, tag="rr_d")
            nc.gpsimd.dma_start(out=rr_d.rearrange("o (c p) -> (o p) c", p=P),
                                in_=rct)
            for ic in range(NI):
                rb = att_s.tile([DV, NIW], FP, tag="rb")
                nc.gpsimd.dma_start(
                    out=rb,
                    in_=rr_d[0:1, ic * NIW:(ic + 1) * NIW].to_broadcast([DV, NIW]))
                nc.vector.tensor_mul(headT[po:po + DK, fc, ic * NIW:(ic + 1) * NIW],
                                     pspvs[ic][0:DV, :], rb)
            prev = (h, rct)
        for pc in range(LC):
            emit_nat_pc(prev[0], prev[1], pc)

    # ---------------- Phase G: output projection + residual + LN ----------------
    with tc.tile_pool(name="ph_g", bufs=3) as pg, \
         tc.tile_pool(name="ph_g_ps", bufs=2, space=MemorySpace.PSUM) as pg_ps:
        for lc in range(LC):
            ps_o = pg_ps.tile([P, D], FP, tag="ps_o")
            for t in range(FC):
                nc.tensor.matmul(ps_o, lhsT=headT[:, t, lc * P:(lc + 1) * P],
                                 rhs=woT[:, t, :], start=(t == 0), stop=(t == FC - 1))
            t1 = pg.tile([P, D], FP, tag="t1")
            nc.vector.tensor_add(t1, ps_o, q_nat[:, lc, :])
            t1b = pg.tile([P, D], FP, tag="t1b")
            nc.vector.tensor_add(t1b, t1, wob_bc)
            stats = pg.tile([P, nc.vector.BN_STATS_DIM], FP, tag="stats")
            nc.vector.bn_stats(out=stats, in_=t1b)
            mv = pg.tile([P, nc.vector.BN_AGGR_DIM], FP, tag="mv")
            nc.vector.bn_aggr(out=mv, in_=stats)
            sd = pg.tile([P, 1], FP, tag="sd")
            nc.scalar.activation(sd, mv[:, 1:2], AF.Sqrt, bias=eps_t)
            rstd = pg.tile([P, 1], FP, tag="rstd")
            nc.vector.reciprocal(rstd, sd)
            t2 = pg.tile([P, D], FP, tag="t2")
            nc.vector.tensor_scalar(out=t2, in0=t1b, scalar1=mv[:, 0:1],
                                    scalar2=rstd, op0=ALU.subtract, op1=ALU.mult)
            t3 = pg.tile([P, D], FP, tag="t3")
            nc.vector.tensor_mul(t3, t2, g_bc)
            t4 = pg.tile([P, D], FP, tag="t4")
            nc.vector.tensor_add(t4, t3, b_bc)
            nc.sync.dma_start(out=out_o[lc * P:(lc + 1) * P, :], in_=t4)


def _build():
    nc = bacc.Bacc()
    t_in = {}
    specs = [
        ("q", [L, D], FP), ("x_", [L, D], BF),
        ("qT", [D, L], BF), ("kT", [D, L], BF), ("vT", [D, L], BF),
        ("Wq", [E, D, F], BF), ("Wk", [E, D, F], BF), ("Wv", [E, D, F], BF),
        ("fc1_w", [E, D], FP), ("fc1_b", [E], FP),
        ("fc2_w", [E, E], FP), ("fc2_b", [E], FP),
        ("woT", [F, D], BF), ("wo_b", [D], FP),
        ("ln_g", [D], FP), ("ln_b", [D], FP),
    ]
    for name, shape, dt in specs:
        t_in[name] = nc.declare_dram_parameter(name, shape, dt, isOutput=False)
    out_o = nc.declare_dram_parameter("out", [L, D], FP, isOutput=True)
    attn_o = nc.declare_dram_parameter("attn", [H, L, L], FP, isOutput=True)
    with tile.TileContext(nc) as tc:
        _body(nc, tc, t_in, out_o, attn_o)
    nc.compile()
    return nc


def _get_nc():
    global _NC
    if _NC is None:
        _NC = _build()
    return _NC


def kernel(q, k, v, x_, Wq, Wk, Wv, fc1_w, fc1_b, fc2_w, fc2_b, wo_w, wo_b,
           ln_g, ln_b, **run_kwargs):
    global LAST_RESULTS
    import ml_dtypes
    BF_NP = ml_dtypes.bfloat16
    nc = _get_nc()
    f32 = lambda a: np.ascontiguousarray(np.asarray(a, dtype=np.float32))
    bf = lambda a: np.ascontiguousarray(np.asarray(a, dtype=np.float32).astype(BF_NP))
    shared = {
        "Wq": bf(Wq), "Wk": bf(Wk), "Wv": bf(Wv),
        "fc1_w": f32(fc1_w), "fc1_b": f32(fc1_b),
        "fc2_w": f32(fc2_w), "fc2_b": f32(fc2_b),
        "woT": bf(np.asarray(wo_w, np.float32).T), "wo_b": f32(wo_b),
        "ln_g": f32(ln_g), "ln_b": f32(ln_b),
    }
    q = f32(q)
    k32, v32, x32 = (np.asarray(a, np.float32) for a in (k, v, x_))
    in_maps = []
    for b in range(B):
        m = dict(shared)
        m["q"] = q[b]
        m["qT"] = bf(q[b].T)
        m["kT"] = bf(k32[b].T)
        m["vT"] = bf(v32[b].T)
        m["x_"] = bf(x32[b])
        in_maps.append(m)
    res = run_bass_kernel_spmd(nc, in_maps, list(range(B)), **run_kwargs)
    LAST_RESULTS = res
    out = np.empty((B, L, D), dtype=np.float32)
    attn = np.empty((H * B, L, L), dtype=np.float32)
    for b in range(B):
        out[b] = res.results[b]["out"]
        a = res.results[b]["attn"]  # [H, L, L]
        for h in range(H):
            attn[h * B + b] = a[h]
    return out, attn


if __name__ == "__main__":
    rng = np.random.default_rng(0)
    ins = {
        "q": rng.standard_normal((B, L, D), dtype=np.float32),
        "k": rng.standard_normal((B, L, D), dtype=np.float32),
        "v": rng.standard_normal((B, L, D), dtype=np.float32),
        "x_": rng.standard_normal((B, L, D), dtype=np.float32),
        "Wq": 0.02 * rng.standard_normal((E, D, F), dtype=np.float32),
        "Wk": 0.02 * rng.standard_normal((E, D, F), dtype=np.float32),
        "Wv": 0.02 * rng.standard_normal((E, D, F), dtype=np.float32),
        "fc1_w": 0.02 * rng.standard_normal((E, D), dtype=np.float32),
        "fc1_b": np.zeros(E, np.float32),
        "fc2_w": 0.02 * rng.standard_normal((E, E), dtype=np.float32),
        "fc2_b": np.zeros(E, np.float32),
        "wo_w": 0.02 * rng.standard_normal((D, F), dtype=np.float32),
        "wo_b": np.zeros(D, np.float32),
        "ln_g": np.ones(D, np.float32),
        "ln_b": np.zeros(D, np.float32),
    }
    out, attn = kernel(**ins)
    print("out", out.shape, out.dtype, "attn", attn.shape, attn.dtype)
    print("attn row sums ~1:", attn.sum(-1).mean())
